# revision 15
# baseline (speedup 1.0000x reference)
"""Trainium2 Bass kernel for nn_AUFusion (dense_mlp, memory-bound).

Reference computation (per sample b):
  feat[b, c]   = sum_k act_c[b, k] * gcn[b, c, k]    act_c = eyebrow (c<3) / mouth (c>=3)
  normed       = LayerNorm(feat) * ln_w + ln_b       (over the 9 features, eps=1e-6)
  out[b, :]    = normed @ lin_w.T + lin_b            (9 -> 5)

Strategy: pure data parallelism, batch 16384 -> 2048 per core on 8 cores.
On-chip layout puts 128 samples on partitions and K=512 on the free axis, so
each of the 9 per-sample dot products is one fused multiply+reduce DVE op
(affine_mul_reduce, a custom-DVE op). Those 144 ops are the only DVE work in
the streaming loop; all LayerNorm + projection math is batched across the 16
sample-tiles into ~10 wide DVE/ACT ops at the end (stride-0 broadcast APs).
The LN affine and Linear layer fold into a single [9,5] matrix W2 / bias b2
on the host.

DMA plan: gcn streams as 16 contiguous ~2.25 MiB tile DMAs on the sync HWDGE
ring (tile 0 split in two so the first DVE op ungates early); the host
pre-interleaves eyebrow/mouth into one partition-major act tensor, loaded in
4 chunks on a 2-slot rotating pool whose WAR dependencies naturally stagger
the later chunks (so act traffic doesn't steal early bandwidth from gcn).
Output is stored in the on-chip [128, 16, 5] layout and transposed on host.
"""

import numpy as np

import concourse.bacc as bacc
import concourse.tile as tile
from concourse import mybir
from concourse.bass_utils import run_bass_kernel_spmd

N_CORES = 8
B = 16384
BPC = B // N_CORES          # samples per core
K = 512
C = 9                       # in features
NCLS = 5                    # num classes
P = 128                     # partitions
NT = BPC // P               # 16 sample-tiles per core
LN_EPS = 1e-6
F32 = mybir.dt.float32

# act chunks: small leading chunks (fast first-compute gate, thin early DMA
# traffic), then 5-tile chunks. The 2-slot pool's WAR deps stagger chunk i
# behind chunk i-2's readers.
ACT_CHUNKS = [(0, 1), (1, 2), (3, 3), (6, 5), (11, 5)]

_NC = None  # built once, reused across calls


def _build_nc():
    nc = bacc.Bacc(None)
    # host-pretransposed, interleaved act: a[p, t*2K + (0:K)] = eyebrow[t*128+p, :],
    # a[p, t*2K + (K:2K)] = mouth[t*128+p, :]
    act = nc.dram_tensor("act", [P, NT * 2 * K], F32, kind="ExternalInput")
    gcn = nc.dram_tensor("gcn", [BPC, C, K], F32, kind="ExternalInput")
    # merged consts: [w2 (NCLS*C) | b2 (NCLS)] broadcast over partitions
    wb = nc.dram_tensor("wb", [P, NCLS * C + NCLS], F32, kind="ExternalInput")
    # [p, t, j] layout; host transposes to [t*128+p, j]
    out = nc.dram_tensor("out", [P, NT * NCLS], F32, kind="ExternalOutput")

    mult = mybir.AluOpType.mult
    add = mybir.AluOpType.add

    with tile.TileContext(nc) as tc:
        with (
            tc.tile_pool(name="gcnp", bufs=6) as gcnp,
            tc.tile_pool(name="actp", bufs=2) as actp,
            tc.tile_pool(name="big", bufs=1) as big,
        ):
            feat = big.tile([P, NT * C], F32)
            dummy = big.tile([P, 1], F32)  # discard target for AMR full out
            wb_sb = big.tile([P, NCLS * C + NCLS], F32)
            nc.scalar.dma_start(wb_sb[:], wb[:])
            w2_sb = wb_sb[:, :NCLS * C].rearrange("p (j c) -> p j c", c=C)
            b2_sb = wb_sb[:, NCLS * C:]
            y = big.tile([P, NT * NCLS], F32)

            def ln_proj(t0, ntl):
                """Batched LayerNorm + projection for tiles [t0, t0+ntl)."""
                f3 = feat[:, t0 * C:(t0 + ntl) * C].rearrange(
                    "p (t c) -> p t c", c=C
                )
                negmu = big.tile([P, ntl], F32, tag=f"negmu{t0}")
                nc.vector.tensor_reduce(
                    out=negmu[:], in_=f3, axis=mybir.AxisListType.X, op=add
                )
                nc.vector.tensor_scalar_mul(negmu[:], negmu[:], -1.0 / C)
                cent = big.tile([P, ntl * C], F32, tag=f"cent{t0}")
                c3 = cent[:].rearrange("p (t c) -> p t c", c=C)
                nc.vector.tensor_tensor(
                    c3, f3, negmu[:][:, :, None].to_broadcast([P, ntl, C]), op=add
                )
                sq = big.tile([P, ntl * C], F32, tag=f"sq{t0}")
                s3 = sq[:].rearrange("p (t c) -> p t c", c=C)
                nc.vector.tensor_tensor(s3, c3, c3, op=mult)
                varp = big.tile([P, ntl], F32, tag=f"varp{t0}")
                nc.vector.tensor_reduce(
                    out=varp[:], in_=s3, axis=mybir.AxisListType.X, op=add
                )
                nc.vector.tensor_scalar(
                    out=varp[:], in0=varp[:], scalar1=1.0 / C, scalar2=LN_EPS,
                    op0=mult, op1=add,
                )
                std = big.tile([P, ntl], F32, tag=f"std{t0}")
                nc.scalar.activation(
                    std[:], varp[:], mybir.ActivationFunctionType.Sqrt
                )
                rstd = big.tile([P, ntl], F32, tag=f"rstd{t0}")
                nc.vector.reciprocal(rstd[:], std[:])
                xhat = big.tile([P, ntl * C], F32, tag=f"xhat{t0}")
                x3 = xhat[:].rearrange("p (t c) -> p t c", c=C)
                nc.vector.tensor_tensor(
                    x3, c3, rstd[:][:, :, None].to_broadcast([P, ntl, C]), op=mult
                )
                prod = big.tile([P, ntl * NCLS * C], F32, tag=f"prod{t0}")
                p4 = prod[:].rearrange("p (t j c) -> p t j c", j=NCLS, c=C)
                nc.vector.tensor_tensor(
                    p4,
                    x3[:, :, None, :].to_broadcast([P, ntl, NCLS, C]),
                    w2_sb[:, None, :, :].to_broadcast([P, ntl, NCLS, C]),
                    op=mult,
                )
                y3 = y[:, t0 * NCLS:(t0 + ntl) * NCLS].rearrange(
                    "p (t j) -> p t j", j=NCLS
                )
                nc.vector.tensor_reduce(
                    out=y3, in_=p4, axis=mybir.AxisListType.X, op=add
                )
                nc.vector.tensor_tensor(
                    y3, y3, b2_sb[:, None, :].to_broadcast([P, ntl, NCLS]), op=add
                )

            def amr(accum_col, g_ap, a_ap):
                nc.vector.affine_mul_reduce(
                    out=dummy.broadcast_to([P, K]),
                    accum_out=feat[:, accum_col:accum_col + 1],
                    in0=g_ap,
                    in1=a_ap,
                    scale=1.0,
                    bias=0.0,
                )

            # ---- streaming loop: 144 fused dot-products on DVE ----
            act_iter = iter(ACT_CHUNKS)
            next_act = next(act_iter)
            cur_act = None
            g0a = big.tile([P, 3 * K], F32)   # tile-0 gcn, eyebrow rows
            g0b = big.tile([P, 6 * K], F32)   # tile-0 gcn, mouth rows
            for t in range(NT):
                if next_act is not None and t == next_act[0]:
                    a0, ntl = next_act
                    a_t = actp.tile([P, ntl * 2 * K], F32, tag="a")
                    if a0 == 0:
                        # split e/m halves so eyebrow AMRs ungate first
                        nc.sync.dma_start(a_t[:, 0:K], act[:, 0:K])
                        nc.sync.dma_start(a_t[:, K:2 * K], act[:, K:2 * K])
                    else:
                        nc.scalar.dma_start(
                            a_t[:], act[:, a0 * 2 * K:(a0 + ntl) * 2 * K]
                        )
                    cur_act = (a_t, a0)
                    next_act = next(act_iter, None)
                a_t, a0 = cur_act
                if t == 0:
                    g3 = gcn[0:P]  # [128, 9, 512]
                    nc.sync.dma_start(
                        g0a[:].rearrange("p (c k) -> p c k", c=3), g3[:, 0:3, :]
                    )
                    nc.sync.dma_start(
                        g0b[:, :3 * K].rearrange("p (c k) -> p c k", c=3),
                        g3[:, 3:6, :],
                    )
                    nc.sync.dma_start(
                        g0b[:, 3 * K:].rearrange("p (c k) -> p c k", c=3),
                        g3[:, 6:9, :],
                    )
                    for c in range(C):
                        g_ap = (
                            g0a[:, c * K:(c + 1) * K]
                            if c < 3
                            else g0b[:, (c - 3) * K:(c - 2) * K]
                        )
                        amr(c, g_ap, a_t[:, (0 if c < 3 else K):(K if c < 3 else 2 * K)])
                    continue
                g_t = gcnp.tile([P, C * K], F32)
                nc.sync.dma_start(
                    g_t[:].rearrange("p (c k) -> p c k", c=C), gcn[t * P:(t + 1) * P]
                )
                for c in range(C):
                    aoff = (t - a0) * 2 * K + (0 if c < 3 else K)
                    amr(t * C + c, g_t[:, c * K:(c + 1) * K], a_t[:, aoff:aoff + K])
                if t == NT - 3:
                    ln_proj(0, NT - 2)   # bulk LN runs in DVE supply gaps

            ln_proj(NT - 2, 2)
            nc.scalar.dma_start(out[:], y[:])

    nc.finalize()
    return nc


def _get_nc():
    global _NC
    if _NC is None:
        _NC = _build_nc()
    return _NC


def _run(inputs, **spmd_kwargs):
    eyebrow = np.ascontiguousarray(inputs["eyebrow"], dtype=np.float32)
    mouth = np.ascontiguousarray(inputs["mouth"], dtype=np.float32)
    gcn = np.ascontiguousarray(inputs["gcn"], dtype=np.float32)
    ln_w = np.asarray(inputs["ln_weight"], dtype=np.float32)
    ln_b = np.asarray(inputs["ln_bias"], dtype=np.float32)
    lin_w = np.asarray(inputs["lin_weight"], dtype=np.float32)
    lin_b = np.asarray(inputs["lin_bias"], dtype=np.float32)

    # Fold LN affine + Linear: normed*ln_w + ln_b then @ lin_w.T + lin_b
    #   == xhat @ W2 + b2 with W2[c,j] = ln_w[c]*lin_w[j,c], b2 = lin_w@ln_b + lin_b
    w2 = (lin_w * ln_w[None, :]).astype(np.float32)        # [NCLS, C] = W2.T
    b2 = (lin_w @ ln_b + lin_b).astype(np.float32)         # [NCLS]
    wb1 = np.concatenate([w2.ravel(), b2]).astype(np.float32)
    wb = np.ascontiguousarray(np.broadcast_to(wb1[None], (P, NCLS * C + NCLS)))

    # per-core partition-major interleaved act layout: [P, NT, 2, K]
    a_sh = np.stack(
        [eyebrow.reshape(N_CORES, NT, P, K), mouth.reshape(N_CORES, NT, P, K)],
        axis=3,
    )  # [cores, NT, P, 2, K]
    a_sh = np.ascontiguousarray(a_sh.transpose(0, 2, 1, 3, 4)).reshape(
        N_CORES, P, NT * 2 * K
    )
    g_sh = gcn.reshape(N_CORES, BPC, C, K)
    in_maps = [
        {"act": a_sh[c], "gcn": g_sh[c], "wb": wb}
        for c in range(N_CORES)
    ]

    res = run_bass_kernel_spmd(
        _get_nc(), in_maps, core_ids=list(range(N_CORES)), **spmd_kwargs
    )
    # out[p, t*5+j] -> full[(core, t*128+p), j]
    out = np.concatenate(
        [
            r["out"].reshape(P, NT, NCLS).transpose(1, 0, 2).reshape(BPC, NCLS)
            for r in res.results
        ],
        axis=0,
    )
    return out, res


def kernel(**inputs):
    out, _ = _run(inputs)
    return out


# revision 16
# speedup vs baseline: 1.0335x; 1.0335x over previous
"""Trainium2 Bass kernel for nn_AUFusion (dense_mlp, memory-bound).

Reference computation (per sample b):
  feat[b, c]   = sum_k act_c[b, k] * gcn[b, c, k]    act_c = eyebrow (c<3) / mouth (c>=3)
  normed       = LayerNorm(feat) * ln_w + ln_b       (over the 9 features, eps=1e-6)
  out[b, :]    = normed @ lin_w.T + lin_b            (9 -> 5)

Strategy: pure data parallelism, batch 16384 -> 2048 per core on 8 cores.
On-chip layout puts 128 samples on partitions and K=512 on the free axis, so
each of the 9 per-sample dot products is one fused multiply+reduce DVE op
(affine_mul_reduce, a custom-DVE op). Those 144 ops are the only DVE work in
the streaming loop; all LayerNorm + projection math is batched across the 16
sample-tiles into ~10 wide DVE/ACT ops at the end (stride-0 broadcast APs).
The LN affine and Linear layer fold into a single [9,5] matrix W2 / bias b2
on the host.

DMA plan: gcn streams as 16 contiguous ~2.25 MiB tile DMAs on the sync HWDGE
ring (tile 0 split in two so the first DVE op ungates early); the host
pre-interleaves eyebrow/mouth into one partition-major act tensor, loaded in
4 chunks on a 2-slot rotating pool whose WAR dependencies naturally stagger
the later chunks (so act traffic doesn't steal early bandwidth from gcn).
Output is stored in the on-chip [128, 16, 5] layout and transposed on host.
"""

import numpy as np

import concourse.bacc as bacc
import concourse.tile as tile
from concourse import mybir
from concourse.bass_utils import run_bass_kernel_spmd

N_CORES = 8
B = 16384
BPC = B // N_CORES          # samples per core
K = 512
C = 9                       # in features
NCLS = 5                    # num classes
P = 128                     # partitions
NT = BPC // P               # 16 sample-tiles per core
LN_EPS = 1e-6
F32 = mybir.dt.float32

# act chunks: small leading chunks (fast first-compute gate, thin early DMA
# traffic), then 5-tile chunks. The 2-slot pool's WAR deps stagger chunk i
# behind chunk i-2's readers.
ACT_CHUNKS = [(0, 1), (1, 2), (3, 3), (6, 5), (11, 5)]

_NC = None  # built once, reused across calls


def _build_nc():
    nc = bacc.Bacc(None)
    # host-pretransposed, interleaved act: a[p, t*2K + (0:K)] = eyebrow[t*128+p, :],
    # a[p, t*2K + (K:2K)] = mouth[t*128+p, :]
    act = nc.dram_tensor("act", [P, NT * 2 * K], F32, kind="ExternalInput")
    gcn = nc.dram_tensor("gcn", [BPC, C, K], F32, kind="ExternalInput")
    # merged consts: [w2 (NCLS*C) | b2 (NCLS)] broadcast over partitions
    wb = nc.dram_tensor("wb", [P, NCLS * C + NCLS], F32, kind="ExternalInput")
    # [p, t, j] layout; host transposes to [t*128+p, j]
    out = nc.dram_tensor("out", [P, NT * NCLS], F32, kind="ExternalOutput")

    mult = mybir.AluOpType.mult
    add = mybir.AluOpType.add

    with tile.TileContext(nc) as tc:
        with (
            tc.tile_pool(name="gcnp", bufs=6) as gcnp,
            tc.tile_pool(name="actp", bufs=2) as actp,
            tc.tile_pool(name="big", bufs=1) as big,
        ):
            feat = big.tile([P, NT * C], F32)
            dummy = big.tile([P, 1], F32)  # discard target for AMR full out
            wb_sb = big.tile([P, NCLS * C + NCLS], F32)
            nc.scalar.dma_start(wb_sb[:], wb[:])
            w2_sb = wb_sb[:, :NCLS * C].rearrange("p (j c) -> p j c", c=C)
            b2_sb = wb_sb[:, NCLS * C:]
            y = big.tile([P, NT * NCLS], F32)

            def ln_proj(t0, ntl):
                """Batched LayerNorm + projection for tiles [t0, t0+ntl)."""
                f3 = feat[:, t0 * C:(t0 + ntl) * C].rearrange(
                    "p (t c) -> p t c", c=C
                )
                negmu = big.tile([P, ntl], F32, tag=f"negmu{t0}")
                nc.vector.tensor_reduce(
                    out=negmu[:], in_=f3, axis=mybir.AxisListType.X, op=add
                )
                nc.vector.tensor_scalar_mul(negmu[:], negmu[:], -1.0 / C)
                cent = big.tile([P, ntl * C], F32, tag=f"cent{t0}")
                c3 = cent[:].rearrange("p (t c) -> p t c", c=C)
                nc.vector.tensor_tensor(
                    c3, f3, negmu[:][:, :, None].to_broadcast([P, ntl, C]), op=add
                )
                sq = big.tile([P, ntl * C], F32, tag=f"sq{t0}")
                s3 = sq[:].rearrange("p (t c) -> p t c", c=C)
                nc.vector.tensor_tensor(s3, c3, c3, op=mult)
                varp = big.tile([P, ntl], F32, tag=f"varp{t0}")
                nc.vector.tensor_reduce(
                    out=varp[:], in_=s3, axis=mybir.AxisListType.X, op=add
                )
                nc.vector.tensor_scalar(
                    out=varp[:], in0=varp[:], scalar1=1.0 / C, scalar2=LN_EPS,
                    op0=mult, op1=add,
                )
                std = big.tile([P, ntl], F32, tag=f"std{t0}")
                nc.scalar.activation(
                    std[:], varp[:], mybir.ActivationFunctionType.Sqrt
                )
                rstd = big.tile([P, ntl], F32, tag=f"rstd{t0}")
                nc.vector.reciprocal(rstd[:], std[:])
                xhat = big.tile([P, ntl * C], F32, tag=f"xhat{t0}")
                x3 = xhat[:].rearrange("p (t c) -> p t c", c=C)
                nc.vector.tensor_tensor(
                    x3, c3, rstd[:][:, :, None].to_broadcast([P, ntl, C]), op=mult
                )
                prod = big.tile([P, ntl * NCLS * C], F32, tag=f"prod{t0}")
                p4 = prod[:].rearrange("p (t j c) -> p t j c", j=NCLS, c=C)
                nc.vector.tensor_tensor(
                    p4,
                    x3[:, :, None, :].to_broadcast([P, ntl, NCLS, C]),
                    w2_sb[:, None, :, :].to_broadcast([P, ntl, NCLS, C]),
                    op=mult,
                )
                y3 = y[:, t0 * NCLS:(t0 + ntl) * NCLS].rearrange(
                    "p (t j) -> p t j", j=NCLS
                )
                nc.vector.tensor_reduce(
                    out=y3, in_=p4, axis=mybir.AxisListType.X, op=add
                )
                nc.vector.tensor_tensor(
                    y3, y3, b2_sb[:, None, :].to_broadcast([P, ntl, NCLS]), op=add
                )

            def amr(accum_col, g_ap, a_ap):
                nc.vector.affine_mul_reduce(
                    out=dummy.broadcast_to([P, K]),
                    accum_out=feat[:, accum_col:accum_col + 1],
                    in0=g_ap,
                    in1=a_ap,
                    scale=1.0,
                    bias=0.0,
                )

            # ---- streaming loop: 144 fused dot-products on DVE ----
            act_iter = iter(ACT_CHUNKS)
            next_act = next(act_iter)
            cur_act = None
            g0a = big.tile([P, 3 * K], F32)   # tile-0 gcn, eyebrow rows
            g0b = big.tile([P, 6 * K], F32)   # tile-0 gcn, mouth rows
            for t in range(NT):
                if next_act is not None and t == next_act[0]:
                    a0, ntl = next_act
                    a_t = actp.tile([P, ntl * 2 * K], F32, tag="a")
                    if a0 == 0:
                        # split e/m halves so eyebrow AMRs ungate first
                        nc.sync.dma_start(a_t[:, 0:K], act[:, 0:K])
                        nc.sync.dma_start(a_t[:, K:2 * K], act[:, K:2 * K])
                    else:
                        nc.scalar.dma_start(
                            a_t[:], act[:, a0 * 2 * K:(a0 + ntl) * 2 * K]
                        )
                    cur_act = (a_t, a0)
                    next_act = next(act_iter, None)
                a_t, a0 = cur_act
                if t == 0:
                    g3 = gcn[0:P]  # [128, 9, 512]
                    nc.sync.dma_start(
                        g0a[:].rearrange("p (c k) -> p c k", c=3), g3[:, 0:3, :]
                    )
                    nc.sync.dma_start(
                        g0b[:, :3 * K].rearrange("p (c k) -> p c k", c=3),
                        g3[:, 3:6, :],
                    )
                    nc.sync.dma_start(
                        g0b[:, 3 * K:].rearrange("p (c k) -> p c k", c=3),
                        g3[:, 6:9, :],
                    )
                    for c in range(C):
                        g_ap = (
                            g0a[:, c * K:(c + 1) * K]
                            if c < 3
                            else g0b[:, (c - 3) * K:(c - 2) * K]
                        )
                        amr(c, g_ap, a_t[:, (0 if c < 3 else K):(K if c < 3 else 2 * K)])
                    continue
                g_t = gcnp.tile([P, C * K], F32)
                nc.sync.dma_start(
                    g_t[:].rearrange("p (c k) -> p c k", c=C), gcn[t * P:(t + 1) * P]
                )
                for c in range(C):
                    aoff = (t - a0) * 2 * K + (0 if c < 3 else K)
                    amr(t * C + c, g_t[:, c * K:(c + 1) * K], a_t[:, aoff:aoff + K])

            ln_proj(0, NT)
            nc.scalar.dma_start(out[:], y[:])

    nc.finalize()
    return nc


def _get_nc():
    global _NC
    if _NC is None:
        _NC = _build_nc()
    return _NC


def _run(inputs, **spmd_kwargs):
    eyebrow = np.ascontiguousarray(inputs["eyebrow"], dtype=np.float32)
    mouth = np.ascontiguousarray(inputs["mouth"], dtype=np.float32)
    gcn = np.ascontiguousarray(inputs["gcn"], dtype=np.float32)
    ln_w = np.asarray(inputs["ln_weight"], dtype=np.float32)
    ln_b = np.asarray(inputs["ln_bias"], dtype=np.float32)
    lin_w = np.asarray(inputs["lin_weight"], dtype=np.float32)
    lin_b = np.asarray(inputs["lin_bias"], dtype=np.float32)

    # Fold LN affine + Linear: normed*ln_w + ln_b then @ lin_w.T + lin_b
    #   == xhat @ W2 + b2 with W2[c,j] = ln_w[c]*lin_w[j,c], b2 = lin_w@ln_b + lin_b
    w2 = (lin_w * ln_w[None, :]).astype(np.float32)        # [NCLS, C] = W2.T
    b2 = (lin_w @ ln_b + lin_b).astype(np.float32)         # [NCLS]
    wb1 = np.concatenate([w2.ravel(), b2]).astype(np.float32)
    wb = np.ascontiguousarray(np.broadcast_to(wb1[None], (P, NCLS * C + NCLS)))

    # per-core partition-major interleaved act layout: [P, NT, 2, K]
    a_sh = np.stack(
        [eyebrow.reshape(N_CORES, NT, P, K), mouth.reshape(N_CORES, NT, P, K)],
        axis=3,
    )  # [cores, NT, P, 2, K]
    a_sh = np.ascontiguousarray(a_sh.transpose(0, 2, 1, 3, 4)).reshape(
        N_CORES, P, NT * 2 * K
    )
    g_sh = gcn.reshape(N_CORES, BPC, C, K)
    in_maps = [
        {"act": a_sh[c], "gcn": g_sh[c], "wb": wb}
        for c in range(N_CORES)
    ]

    res = run_bass_kernel_spmd(
        _get_nc(), in_maps, core_ids=list(range(N_CORES)), **spmd_kwargs
    )
    # out[p, t*5+j] -> full[(core, t*128+p), j]
    out = np.concatenate(
        [
            r["out"].reshape(P, NT, NCLS).transpose(1, 0, 2).reshape(BPC, NCLS)
            for r in res.results
        ],
        axis=0,
    )
    return out, res


def kernel(**inputs):
    out, _ = _run(inputs)
    return out


# revision 17
# speedup vs baseline: 1.0691x; 1.0344x over previous
"""Trainium2 Bass kernel for nn_AUFusion (dense_mlp, memory-bound).

Reference computation (per sample b):
  feat[b, c]   = sum_k act_c[b, k] * gcn[b, c, k]    act_c = eyebrow (c<3) / mouth (c>=3)
  normed       = LayerNorm(feat) * ln_w + ln_b       (over the 9 features, eps=1e-6)
  out[b, :]    = normed @ lin_w.T + lin_b            (9 -> 5)

Strategy: pure data parallelism, batch 16384 -> 2048 per core on 8 cores.
On-chip layout puts 128 samples on partitions and K=512 on the free axis, so
each of the 9 per-sample dot products is one fused multiply+reduce DVE op
(affine_mul_reduce, a custom-DVE op). Those 144 ops are the only DVE work in
the streaming loop; all LayerNorm + projection math is batched across the 16
sample-tiles into ~10 wide DVE/ACT ops at the end (stride-0 broadcast APs).
The LN affine and Linear layer fold into a single [9,5] matrix W2 / bias b2
on the host.

DMA plan: gcn streams as 16 contiguous ~2.25 MiB tile DMAs on the sync HWDGE
ring (tile 0 split in two so the first DVE op ungates early); the host
pre-interleaves eyebrow/mouth into one partition-major act tensor, loaded in
4 chunks on a 2-slot rotating pool whose WAR dependencies naturally stagger
the later chunks (so act traffic doesn't steal early bandwidth from gcn).
Output is stored in the on-chip [128, 16, 5] layout and transposed on host.
"""

import numpy as np

import concourse.bacc as bacc
import concourse.tile as tile
from concourse import mybir
from concourse.bass_utils import run_bass_kernel_spmd

N_CORES = 8
B = 16384
BPC = B // N_CORES          # samples per core
K = 512
C = 9                       # in features
NCLS = 5                    # num classes
P = 128                     # partitions
NT = BPC // P               # 16 sample-tiles per core
LN_EPS = 1e-6
F32 = mybir.dt.float32

# act chunks: small leading chunks (fast first-compute gate, thin early DMA
# traffic), then 5-tile chunks. The 2-slot pool's WAR deps stagger chunk i
# behind chunk i-2's readers.
ACT_CHUNKS = [(0, 1), (1, 2), (3, 3), (6, 5), (11, 5)]

_NC = None  # built once, reused across calls


def _build_nc():
    nc = bacc.Bacc(None)
    # host-pretransposed, interleaved act: a[p, t*2K + (0:K)] = eyebrow[t*128+p, :],
    # a[p, t*2K + (K:2K)] = mouth[t*128+p, :]
    act = nc.dram_tensor("act", [P, NT * 2 * K], F32, kind="ExternalInput")
    gcn = nc.dram_tensor("gcn", [BPC, C, K], F32, kind="ExternalInput")
    # merged consts: [w2 (NCLS*C) | b2 (NCLS)] broadcast over partitions
    wb = nc.dram_tensor("wb", [P, NCLS * C + NCLS], F32, kind="ExternalInput")
    # [p, t, j] layout; host transposes to [t*128+p, j]
    out = nc.dram_tensor("out", [P, NT * NCLS], F32, kind="ExternalOutput")

    mult = mybir.AluOpType.mult
    add = mybir.AluOpType.add

    with tile.TileContext(nc) as tc:
        with (
            tc.tile_pool(name="gcnp", bufs=6) as gcnp,
            tc.tile_pool(name="actp", bufs=2) as actp,
            tc.tile_pool(name="big", bufs=1) as big,
        ):
            feat = big.tile([P, NT * C], F32)
            dummy = big.tile([P, 1], F32)  # discard target for AMR full out
            wb_sb = big.tile([P, NCLS * C + NCLS], F32)
            w2_sb = wb_sb[:, :NCLS * C].rearrange("p (j c) -> p j c", c=C)
            b2_sb = wb_sb[:, NCLS * C:]
            y = big.tile([P, NT * NCLS], F32)

            def ln_proj(t0, ntl):
                """Batched LayerNorm + projection for tiles [t0, t0+ntl)."""
                f3 = feat[:, t0 * C:(t0 + ntl) * C].rearrange(
                    "p (t c) -> p t c", c=C
                )
                negmu = big.tile([P, ntl], F32, tag=f"negmu{t0}")
                nc.vector.tensor_reduce(
                    out=negmu[:], in_=f3, axis=mybir.AxisListType.X, op=add
                )
                nc.vector.tensor_scalar_mul(negmu[:], negmu[:], -1.0 / C)
                cent = big.tile([P, ntl * C], F32, tag=f"cent{t0}")
                c3 = cent[:].rearrange("p (t c) -> p t c", c=C)
                nc.vector.tensor_tensor(
                    c3, f3, negmu[:][:, :, None].to_broadcast([P, ntl, C]), op=add
                )
                sq = big.tile([P, ntl * C], F32, tag=f"sq{t0}")
                s3 = sq[:].rearrange("p (t c) -> p t c", c=C)
                nc.vector.tensor_tensor(s3, c3, c3, op=mult)
                varp = big.tile([P, ntl], F32, tag=f"varp{t0}")
                nc.vector.tensor_reduce(
                    out=varp[:], in_=s3, axis=mybir.AxisListType.X, op=add
                )
                nc.vector.tensor_scalar(
                    out=varp[:], in0=varp[:], scalar1=1.0 / C, scalar2=LN_EPS,
                    op0=mult, op1=add,
                )
                std = big.tile([P, ntl], F32, tag=f"std{t0}")
                nc.scalar.activation(
                    std[:], varp[:], mybir.ActivationFunctionType.Sqrt
                )
                rstd = big.tile([P, ntl], F32, tag=f"rstd{t0}")
                nc.vector.reciprocal(rstd[:], std[:])
                xhat = big.tile([P, ntl * C], F32, tag=f"xhat{t0}")
                x3 = xhat[:].rearrange("p (t c) -> p t c", c=C)
                nc.vector.tensor_tensor(
                    x3, c3, rstd[:][:, :, None].to_broadcast([P, ntl, C]), op=mult
                )
                prod = big.tile([P, ntl * NCLS * C], F32, tag=f"prod{t0}")
                p4 = prod[:].rearrange("p (t j c) -> p t j c", j=NCLS, c=C)
                nc.vector.tensor_tensor(
                    p4,
                    x3[:, :, None, :].to_broadcast([P, ntl, NCLS, C]),
                    w2_sb[:, None, :, :].to_broadcast([P, ntl, NCLS, C]),
                    op=mult,
                )
                y3 = y[:, t0 * NCLS:(t0 + ntl) * NCLS].rearrange(
                    "p (t j) -> p t j", j=NCLS
                )
                nc.vector.tensor_reduce(
                    out=y3, in_=p4, axis=mybir.AxisListType.X, op=add
                )
                nc.vector.tensor_tensor(
                    y3, y3, b2_sb[:, None, :].to_broadcast([P, ntl, NCLS]), op=add
                )

            def amr(accum_col, g_ap, a_ap):
                nc.vector.affine_mul_reduce(
                    out=dummy.broadcast_to([P, K]),
                    accum_out=feat[:, accum_col:accum_col + 1],
                    in0=g_ap,
                    in1=a_ap,
                    scale=1.0,
                    bias=0.0,
                )

            # ---- streaming loop: 144 fused dot-products on DVE ----
            act_iter = iter(ACT_CHUNKS)
            next_act = next(act_iter)
            cur_act = None
            g0a = big.tile([P, 3 * K], F32)   # tile-0 gcn, eyebrow rows
            g0b = big.tile([P, 6 * K], F32)   # tile-0 gcn, mouth rows
            for t in range(NT):
                if next_act is not None and t == next_act[0]:
                    a0, ntl = next_act
                    a_t = actp.tile([P, ntl * 2 * K], F32, tag="a")
                    if a0 == 0:
                        # split e/m halves so eyebrow AMRs ungate first
                        nc.sync.dma_start(a_t[:, 0:K], act[:, 0:K])
                        nc.sync.dma_start(a_t[:, K:2 * K], act[:, K:2 * K])
                    else:
                        nc.scalar.dma_start(
                            a_t[:], act[:, a0 * 2 * K:(a0 + ntl) * 2 * K]
                        )
                    cur_act = (a_t, a0)
                    next_act = next(act_iter, None)
                a_t, a0 = cur_act
                if t == 0:
                    g3 = gcn[0:P]  # [128, 9, 512]
                    nc.sync.dma_start(
                        g0a[:].rearrange("p (c k) -> p c k", c=3), g3[:, 0:3, :]
                    )
                    nc.sync.dma_start(
                        g0b[:, :3 * K].rearrange("p (c k) -> p c k", c=3),
                        g3[:, 3:6, :],
                    )
                    nc.sync.dma_start(
                        g0b[:, 3 * K:].rearrange("p (c k) -> p c k", c=3),
                        g3[:, 6:9, :],
                    )
                    for c in range(C):
                        g_ap = (
                            g0a[:, c * K:(c + 1) * K]
                            if c < 3
                            else g0b[:, (c - 3) * K:(c - 2) * K]
                        )
                        amr(c, g_ap, a_t[:, (0 if c < 3 else K):(K if c < 3 else 2 * K)])
                    continue
                g_t = gcnp.tile([P, C * K], F32)
                nc.sync.dma_start(
                    g_t[:].rearrange("p (c k) -> p c k", c=C), gcn[t * P:(t + 1) * P]
                )
                for c in range(C):
                    aoff = (t - a0) * 2 * K + (0 if c < 3 else K)
                    amr(t * C + c, g_t[:, c * K:(c + 1) * K], a_t[:, aoff:aoff + K])

            nc.scalar.dma_start(wb_sb[:], wb[:])
            ln_proj(0, NT)
            nc.scalar.dma_start(out[:], y[:])

    nc.finalize()
    return nc


def _get_nc():
    global _NC
    if _NC is None:
        _NC = _build_nc()
    return _NC


def _run(inputs, **spmd_kwargs):
    eyebrow = np.ascontiguousarray(inputs["eyebrow"], dtype=np.float32)
    mouth = np.ascontiguousarray(inputs["mouth"], dtype=np.float32)
    gcn = np.ascontiguousarray(inputs["gcn"], dtype=np.float32)
    ln_w = np.asarray(inputs["ln_weight"], dtype=np.float32)
    ln_b = np.asarray(inputs["ln_bias"], dtype=np.float32)
    lin_w = np.asarray(inputs["lin_weight"], dtype=np.float32)
    lin_b = np.asarray(inputs["lin_bias"], dtype=np.float32)

    # Fold LN affine + Linear: normed*ln_w + ln_b then @ lin_w.T + lin_b
    #   == xhat @ W2 + b2 with W2[c,j] = ln_w[c]*lin_w[j,c], b2 = lin_w@ln_b + lin_b
    w2 = (lin_w * ln_w[None, :]).astype(np.float32)        # [NCLS, C] = W2.T
    b2 = (lin_w @ ln_b + lin_b).astype(np.float32)         # [NCLS]
    wb1 = np.concatenate([w2.ravel(), b2]).astype(np.float32)
    wb = np.ascontiguousarray(np.broadcast_to(wb1[None], (P, NCLS * C + NCLS)))

    # per-core partition-major interleaved act layout: [P, NT, 2, K]
    a_sh = np.stack(
        [eyebrow.reshape(N_CORES, NT, P, K), mouth.reshape(N_CORES, NT, P, K)],
        axis=3,
    )  # [cores, NT, P, 2, K]
    a_sh = np.ascontiguousarray(a_sh.transpose(0, 2, 1, 3, 4)).reshape(
        N_CORES, P, NT * 2 * K
    )
    g_sh = gcn.reshape(N_CORES, BPC, C, K)
    in_maps = [
        {"act": a_sh[c], "gcn": g_sh[c], "wb": wb}
        for c in range(N_CORES)
    ]

    res = run_bass_kernel_spmd(
        _get_nc(), in_maps, core_ids=list(range(N_CORES)), **spmd_kwargs
    )
    # out[p, t*5+j] -> full[(core, t*128+p), j]
    out = np.concatenate(
        [
            r["out"].reshape(P, NT, NCLS).transpose(1, 0, 2).reshape(BPC, NCLS)
            for r in res.results
        ],
        axis=0,
    )
    return out, res


def kernel(**inputs):
    out, _ = _run(inputs)
    return out


# revision 22
# speedup vs baseline: 1.1629x; 1.0878x over previous
"""Trainium2 Bass kernel for nn_AUFusion (dense_mlp, memory-bound).

Reference computation (per sample b):
  feat[b, c]   = sum_k act_c[b, k] * gcn[b, c, k]    act_c = eyebrow (c<3) / mouth (c>=3)
  normed       = LayerNorm(feat) * ln_w + ln_b       (over the 9 features, eps=1e-6)
  out[b, :]    = normed @ lin_w.T + lin_b            (9 -> 5)

Strategy: pure data parallelism, batch 16384 -> 2048 per core on 8 cores.
On-chip layout puts 128 samples on partitions and K=512 on the free axis, so
each of the 9 per-sample dot products is one fused multiply+reduce DVE op
(affine_mul_reduce, a custom-DVE op). Those 144 ops are the only DVE work in
the streaming loop; all LayerNorm + projection math is batched across the 16
sample-tiles into ~10 wide DVE/ACT ops at the end (stride-0 broadcast APs).
The LN affine and Linear layer fold into a single [9,5] matrix W2 / bias b2
on the host.

DMA plan: gcn streams as 16 contiguous ~2.25 MiB tile DMAs on the sync HWDGE
ring (tile 0 split in two so the first DVE op ungates early); the host
pre-interleaves eyebrow/mouth into one partition-major act tensor, loaded in
4 chunks on a 2-slot rotating pool whose WAR dependencies naturally stagger
the later chunks (so act traffic doesn't steal early bandwidth from gcn).
Output is stored in the on-chip [128, 16, 5] layout and transposed on host.
"""

import numpy as np

import concourse.bacc as bacc
import concourse.tile as tile
from concourse import mybir
from concourse.bass_utils import run_bass_kernel_spmd

N_CORES = 8
B = 16384
BPC = B // N_CORES          # samples per core
K = 512
C = 9                       # in features
NCLS = 5                    # num classes
P = 128                     # partitions
NT = BPC // P               # 16 sample-tiles per core
LN_EPS = 1e-6
F32 = mybir.dt.float32

# act chunks: small leading chunks (fast first-compute gate, thin early DMA
# traffic), then 5-tile chunks. The 2-slot pool's WAR deps stagger chunk i
# behind chunk i-2's readers.
ACT_CHUNKS = [(0, 1), (1, 2), (3, 3), (6, 5), (11, 5)]

_NC = None  # built once, reused across calls


def _build_nc():
    nc = bacc.Bacc(None)
    # host-pretransposed, interleaved act: a[p, t*2K + (0:K)] = eyebrow[t*128+p, :],
    # a[p, t*2K + (K:2K)] = mouth[t*128+p, :]
    act = nc.dram_tensor("act", [P, NT * 2 * K], F32, kind="ExternalInput")
    gcn = nc.dram_tensor("gcn", [BPC, C, K], F32, kind="ExternalInput")
    # merged consts: [w2 (NCLS*C) | b2 (NCLS)] broadcast over partitions
    wb = nc.dram_tensor("wb", [P, NCLS * C + NCLS], F32, kind="ExternalInput")
    # [p, t, j] layout; host transposes to [t*128+p, j]
    out = nc.dram_tensor("out", [P, NT * NCLS], F32, kind="ExternalOutput")

    mult = mybir.AluOpType.mult
    add = mybir.AluOpType.add

    with tile.TileContext(nc) as tc:
        with (
            tc.tile_pool(name="gcnp", bufs=6) as gcnp,
            tc.tile_pool(name="actp", bufs=3) as actp,
            tc.tile_pool(name="big", bufs=1) as big,
        ):
            feat = big.tile([P, NT * C], F32)
            dummy = big.tile([P, 1], F32)  # discard target for AMR full out
            wb_sb = big.tile([P, NCLS * C + NCLS], F32)
            w2_sb = wb_sb[:, :NCLS * C].rearrange("p (j c) -> p j c", c=C)
            b2_sb = wb_sb[:, NCLS * C:]
            y = big.tile([P, NT * NCLS], F32)

            def ln_proj(t0, ntl):
                """Batched LayerNorm + projection for tiles [t0, t0+ntl)."""
                f3 = feat[:, t0 * C:(t0 + ntl) * C].rearrange(
                    "p (t c) -> p t c", c=C
                )
                negmu = big.tile([P, ntl], F32, tag=f"negmu{t0}")
                nc.vector.tensor_reduce(
                    out=negmu[:], in_=f3, axis=mybir.AxisListType.X, op=add
                )
                nc.vector.tensor_scalar_mul(negmu[:], negmu[:], -1.0 / C)
                cent = big.tile([P, ntl * C], F32, tag=f"cent{t0}")
                c3 = cent[:].rearrange("p (t c) -> p t c", c=C)
                nc.vector.tensor_tensor(
                    c3, f3, negmu[:][:, :, None].to_broadcast([P, ntl, C]), op=add
                )
                sq = big.tile([P, ntl * C], F32, tag=f"sq{t0}")
                s3 = sq[:].rearrange("p (t c) -> p t c", c=C)
                nc.vector.tensor_tensor(s3, c3, c3, op=mult)
                varp = big.tile([P, ntl], F32, tag=f"varp{t0}")
                nc.vector.tensor_reduce(
                    out=varp[:], in_=s3, axis=mybir.AxisListType.X, op=add
                )
                nc.vector.tensor_scalar(
                    out=varp[:], in0=varp[:], scalar1=1.0 / C, scalar2=LN_EPS,
                    op0=mult, op1=add,
                )
                std = big.tile([P, ntl], F32, tag=f"std{t0}")
                nc.scalar.activation(
                    std[:], varp[:], mybir.ActivationFunctionType.Sqrt
                )
                rstd = big.tile([P, ntl], F32, tag=f"rstd{t0}")
                nc.vector.reciprocal(rstd[:], std[:])
                xhat = big.tile([P, ntl * C], F32, tag=f"xhat{t0}")
                x3 = xhat[:].rearrange("p (t c) -> p t c", c=C)
                nc.vector.tensor_tensor(
                    x3, c3, rstd[:][:, :, None].to_broadcast([P, ntl, C]), op=mult
                )
                prod = big.tile([P, ntl * NCLS * C], F32, tag=f"prod{t0}")
                p4 = prod[:].rearrange("p (t j c) -> p t j c", j=NCLS, c=C)
                nc.vector.tensor_tensor(
                    p4,
                    x3[:, :, None, :].to_broadcast([P, ntl, NCLS, C]),
                    w2_sb[:, None, :, :].to_broadcast([P, ntl, NCLS, C]),
                    op=mult,
                )
                y3 = y[:, t0 * NCLS:(t0 + ntl) * NCLS].rearrange(
                    "p (t j) -> p t j", j=NCLS
                )
                nc.vector.tensor_reduce(
                    out=y3, in_=p4, axis=mybir.AxisListType.X, op=add
                )
                nc.vector.tensor_tensor(
                    y3, y3, b2_sb[:, None, :].to_broadcast([P, ntl, NCLS]), op=add
                )

            def amr(accum_col, g_ap, a_ap):
                nc.vector.affine_mul_reduce(
                    out=dummy.broadcast_to([P, K]),
                    accum_out=feat[:, accum_col:accum_col + 1],
                    in0=g_ap,
                    in1=a_ap,
                    scale=1.0,
                    bias=0.0,
                )

            # ---- streaming loop: 144 fused dot-products on DVE ----
            act_iter = iter(ACT_CHUNKS)
            next_act = next(act_iter)
            cur_act = None
            g0a = big.tile([P, 3 * K], F32)   # tile-0 gcn, eyebrow rows
            g0b = big.tile([P, 6 * K], F32)   # tile-0 gcn, mouth rows
            for t in range(NT):
                if next_act is not None and t == next_act[0]:
                    a0, ntl = next_act
                    a_t = actp.tile([P, ntl * 2 * K], F32, tag="a")
                    if a0 == 0:
                        # split e/m halves so eyebrow AMRs ungate first
                        g3 = gcn[0:P]  # [128, 9, 512]
                        nc.sync.dma_start(a_t[:, 0:K], act[:, 0:K])
                        nc.sync.dma_start(
                            g0a[:].rearrange("p (c k) -> p c k", c=3),
                            g3[:, 0:3, :],
                        )
                        nc.sync.dma_start(a_t[:, K:2 * K], act[:, K:2 * K])
                        nc.sync.dma_start(
                            g0b[:, :3 * K].rearrange("p (c k) -> p c k", c=3),
                            g3[:, 3:6, :],
                        )
                        nc.sync.dma_start(
                            g0b[:, 3 * K:].rearrange("p (c k) -> p c k", c=3),
                            g3[:, 6:9, :],
                        )
                    else:
                        nc.scalar.dma_start(
                            a_t[:], act[:, a0 * 2 * K:(a0 + ntl) * 2 * K]
                        )
                    cur_act = (a_t, a0)
                    next_act = next(act_iter, None)
                a_t, a0 = cur_act
                if t == 0:
                    for c in range(C):
                        g_ap = (
                            g0a[:, c * K:(c + 1) * K]
                            if c < 3
                            else g0b[:, (c - 3) * K:(c - 2) * K]
                        )
                        amr(c, g_ap, a_t[:, (0 if c < 3 else K):(K if c < 3 else 2 * K)])
                    continue
                g_t = gcnp.tile([P, C * K], F32)
                nc.sync.dma_start(
                    g_t[:].rearrange("p (c k) -> p c k", c=C), gcn[t * P:(t + 1) * P]
                )
                for c in range(C):
                    aoff = (t - a0) * 2 * K + (0 if c < 3 else K)
                    amr(t * C + c, g_t[:, c * K:(c + 1) * K], a_t[:, aoff:aoff + K])

            nc.scalar.dma_start(wb_sb[:], wb[:])
            ln_proj(0, NT)
            nc.scalar.dma_start(out[:], y[:])

    nc.finalize()
    return nc


def _get_nc():
    global _NC
    if _NC is None:
        _NC = _build_nc()
    return _NC


def _run(inputs, **spmd_kwargs):
    eyebrow = np.ascontiguousarray(inputs["eyebrow"], dtype=np.float32)
    mouth = np.ascontiguousarray(inputs["mouth"], dtype=np.float32)
    gcn = np.ascontiguousarray(inputs["gcn"], dtype=np.float32)
    ln_w = np.asarray(inputs["ln_weight"], dtype=np.float32)
    ln_b = np.asarray(inputs["ln_bias"], dtype=np.float32)
    lin_w = np.asarray(inputs["lin_weight"], dtype=np.float32)
    lin_b = np.asarray(inputs["lin_bias"], dtype=np.float32)

    # Fold LN affine + Linear: normed*ln_w + ln_b then @ lin_w.T + lin_b
    #   == xhat @ W2 + b2 with W2[c,j] = ln_w[c]*lin_w[j,c], b2 = lin_w@ln_b + lin_b
    w2 = (lin_w * ln_w[None, :]).astype(np.float32)        # [NCLS, C] = W2.T
    b2 = (lin_w @ ln_b + lin_b).astype(np.float32)         # [NCLS]
    wb1 = np.concatenate([w2.ravel(), b2]).astype(np.float32)
    wb = np.ascontiguousarray(np.broadcast_to(wb1[None], (P, NCLS * C + NCLS)))

    # per-core partition-major interleaved act layout: [P, NT, 2, K]
    a_sh = np.stack(
        [eyebrow.reshape(N_CORES, NT, P, K), mouth.reshape(N_CORES, NT, P, K)],
        axis=3,
    )  # [cores, NT, P, 2, K]
    a_sh = np.ascontiguousarray(a_sh.transpose(0, 2, 1, 3, 4)).reshape(
        N_CORES, P, NT * 2 * K
    )
    g_sh = gcn.reshape(N_CORES, BPC, C, K)
    in_maps = [
        {"act": a_sh[c], "gcn": g_sh[c], "wb": wb}
        for c in range(N_CORES)
    ]

    res = run_bass_kernel_spmd(
        _get_nc(), in_maps, core_ids=list(range(N_CORES)), **spmd_kwargs
    )
    # out[p, t*5+j] -> full[(core, t*128+p), j]
    out = np.concatenate(
        [
            r["out"].reshape(P, NT, NCLS).transpose(1, 0, 2).reshape(BPC, NCLS)
            for r in res.results
        ],
        axis=0,
    )
    return out, res


def kernel(**inputs):
    out, _ = _run(inputs)
    return out
